# revision 38
# baseline (speedup 1.0000x reference)
"""Dempster-Shafer evidential module on 8 Trainium2 cores.

Math: the reference's per-step Dempster normalization cancels, so the scan
collapses to an affine recurrence per (batch b, class k):

    z_t = shat[b,t,k]*z_{t-1} + 2/3,   z after prototype 0 = 1 + u[k,0]*rho[b,0]
    shat = 1/3 + (u/3)*rho,  rho = si/(maxsi + 1e-4 - si),  si = exp(T)
    T[p,b] = 2g x.w_p - g|w_p|^2 + ln a - g|x|^2
    y = z_T - 1;  out[b,k] = y/(sum_k y + 1);  out[b,C] = 1/(sum_k y + 1)

v2 structure (43.4us -> target ~32us; TimelineSim cost model):
  - DVE reduced to (nearly) scans only: the reciprocal chain moved off DVE
    via rec = Exp(-Ln(dent)) on Act (error ~1e-6, below the f32r rho
    quantization).  dent on Pool (2 ops) for slices 2+; slices 0-1 keep the
    short all-DVE chain (dent STT -> rec approx -> rho mult) since DVE is
    idle pre-stream anyway.
  - ONPE chunks: +1/3 comes from ONE K=2 matmul pass (crow2 as [2, QN]
    f32r a+b pair rows); the scan reads qs straight from PSUM (+65ns/scan).
    Remaining chunks use the Act evac as before.  Balances Act vs PE.
  - finals: reduce on Pool, tiny add/reciprocal on DVE, out-writes on Act
    (Copy scale=drq bias=-drq); last chunk (m=15) runs a minimal all-DVE
    chain reading the scan output directly to shorten the tail.
  - DMA: crowb/x0/wh/cb2/crow2 early on HWDGE, ublk split into 3 pieces,
    xs1 via the gpsimd SWDGE queue in parallel.
"""

import numpy as np

B, F, P, C = 16384, 512, 128, 10
NCORES = 8
BL = B // NCORES          # 2048 rows per core
SEG = P + 1               # 129 columns per class segment
QN = C * SEG              # 1290 scan columns
OUTW = 16 * (C + 1)       # 176 packed output columns

# batch-column slices (start, ncols); chunk m = col/128, 16 chunks total
SLICES = [(0, 128), (128, 128), (256, 256), (512, 512), (1024, 384),
          (1408, 512), (1920, 128)]
ONPE = (0, 2, 4, 6, 8, 10, 12, 14)   # chunks: +1/3 via K=2 crow2 matmul,
                                      # scan reads PSUM (no Act evac)
# per-slice rho-chain mode: "dve" = dent/rec/rho all DVE (pre-stream window)
# "lnexp" = dent on Pool, rec = Exp(-Ln(dent)) on Act, rho mult on Pool
RHO_MODE = ("dve", "dve", "mix", "mix", "mix", "mix", "mix")
DMA_ORDER = ["wh", "xs0", "cb2", "crowb", "crow2p", "ublkAp", "ublkBp", "xs1",
             "xs23", "ublkC", "x47", "x811", "x1215"]
FINDVE = (15,)            # chunks with the minimal all-DVE finals tail
BATCH_Q3 = True
LINEARIZE = False
SCAN_BUFS = 3
POOL_MODE = "stack"
SI_BUFS = 4
RHO_BUFS = 5
PIPE_LAG = 3
FIN_REDUCE_POOL = False   # gpsimd tensor_reduce is partition-axis only
WARMUP = 2

_PROG = {}
REPS = 1


def _build_program():
    import concourse.bacc as bacc
    import concourse.bass as bass
    import concourse.tile as tile
    from concourse import bass_isa, mybir

    f32 = mybir.dt.float32
    f16 = mybir.dt.float16
    f32r = mybir.dt.float32r
    Alu = mybir.AluOpType
    Act = mybir.ActivationFunctionType

    nc = bacc.Bacc("TRN2", target_bir_lowering=False, debug=False)

    # x slice-contiguous: 128-col block (slice s, chunk c) at col s*512+c*128
    xq_d = nc.dram_tensor("xq", [128, 8192], f16, kind="ExternalInput").ap()
    wh_d = nc.dram_tensor("whp", [128, 1024], f16, kind="ExternalInput").ap()
    cb2_d = nc.dram_tensor("cb2", [2, BL], f32r, kind="ExternalInput").ap()
    crowb_d = nc.dram_tensor("crowb", [128, 1], f32, kind="ExternalInput").ap()
    ublk_d = nc.dram_tensor("ublk", [P, QN], f16, kind="ExternalInput").ap()
    c2ab_d = nc.dram_tensor("crow2ab", [2, QN], f32r, kind="ExternalInput").ap()
    out_d = nc.dram_tensor("out", [128, OUTW], f32, kind="ExternalOutput").ap()

    with tile.TileContext(nc, linearize=LINEARIZE,
                          pool_alloc_mode=POOL_MODE) as tc:
        for _rep in range(REPS):
            with (
                tc.tile_pool(name="const", bufs=1) as cpool,
                tc.tile_pool(name="xin", bufs=1) as xpool,
                tc.tile_pool(name="mid", bufs=1) as mpool,
                tc.tile_pool(name="scan", bufs=SCAN_BUFS) as spool,
                tc.tile_pool(name="pst", bufs=2, space=bass.MemorySpace.PSUM) as pst,
                tc.tile_pool(name="pq", bufs=2, space=bass.MemorySpace.PSUM) as pq,
            ):
                # ---- input DMAs (HWDGE serializes at 625ns/descriptor; DMA
                #      transfers serialize at ~360B/ns: order = startup path) ----
                xall = xpool.tile([128, 8192], f16, tag="xall")
                wh = cpool.tile([128, 1024], f16, tag="wh")
                cb2 = cpool.tile([2, BL], f32r, tag="cb2")
                crowb = cpool.tile([128, 1], f32, tag="crowb")
                ublk = cpool.tile([P, QN], f16, tag="ublk")
                crow2 = cpool.tile([2, QN], f32r, tag="crow2")

                dmas = {
                    "xs0": lambda: nc.sync.dma_start(xall[:, 0:512], xq_d[:, 0:512]),
                    "xs0p": lambda: nc.gpsimd.dma_start(xall[:, 0:512], xq_d[:, 0:512]),
                    "wh": lambda: nc.sync.dma_start(wh[:], wh_d[:]),
                    "whp": lambda: nc.gpsimd.dma_start(wh[:], wh_d[:]),
                    "cb2": lambda: nc.sync.dma_start(cb2[:], cb2_d[:]),
                    "cb2p": lambda: nc.gpsimd.dma_start(cb2[:], cb2_d[:]),
                    "crowbp": lambda: nc.gpsimd.dma_start(crowb[:], crowb_d[:]),
                    "crow2p": lambda: nc.gpsimd.dma_start(crow2[:], c2ab_d[:]),
                    "crowb": lambda: nc.sync.dma_start(crowb[:], crowb_d[:]),
                    "crow2": lambda: nc.sync.dma_start(crow2[:], c2ab_d[:]),
                    "ublk": lambda: nc.sync.dma_start(ublk[:], ublk_d[:]),
                    "ublkp": lambda: nc.gpsimd.dma_start(ublk[:], ublk_d[:]),
                    "ublkA": lambda: nc.sync.dma_start(ublk[:, 0:512], ublk_d[:, 0:512]),
                    "ublkAp": lambda: nc.gpsimd.dma_start(ublk[:, 0:512], ublk_d[:, 0:512]),
                    "ublkBp": lambda: nc.gpsimd.dma_start(ublk[:, 512:1024], ublk_d[:, 512:1024]),
                    "ublkB": lambda: nc.sync.dma_start(ublk[:, 512:1024], ublk_d[:, 512:1024]),
                    "ublkC": lambda: nc.sync.dma_start(ublk[:, 1024:QN], ublk_d[:, 1024:QN]),
                    "xs1": lambda: nc.sync.dma_start(xall[:, 512:1024], xq_d[:, 512:1024]),
                    "xs1p": lambda: nc.gpsimd.dma_start(xall[:, 512:1024], xq_d[:, 512:1024]),
                    "xs23": lambda: nc.sync.dma_start(xall[:, 1024:2048], xq_d[:, 1024:2048]),
                    "x47": lambda: nc.sync.dma_start(xall[:, 2048:4096], xq_d[:, 2048:4096]),
                    "x811": lambda: nc.sync.dma_start(xall[:, 4096:6144], xq_d[:, 4096:6144]),
                    "x1215": lambda: nc.sync.dma_start(xall[:, 6144:8192], xq_d[:, 6144:8192]),
                }
                # ---- device-built constants (memsets precede the Pool
                #      SWDGE issues so nothing queues behind them) ----
                ones2f = cpool.tile([2, 128], f32, tag="ones2")
                nc.gpsimd.memset(ones2f[:], 1.0)
                ones2 = ones2f[:].bitcast(f32r)
                data1 = cpool.tile([128, QN], f32, tag="data1")
                nc.gpsimd.memset(data1[:], 2.0 / 3.0)
                d1v = data1[:].rearrange("p (k s) -> p k s", s=SEG)
                nc.gpsimd.memset(d1v[:, :, 0], 1.0)

                for k in DMA_ORDER:
                    dmas[k]()
                actw = cpool.tile([2, 1], f32, tag="actw")
                nc.scalar.activation(actw[:], ones2f[:, 0:1], Act.Exp)

                nb9 = cpool.tile([128, 1], f32, tag="nb9")
                nc.gpsimd.memset(nb9[:], -(C - 1.0) / C)
                warmb = cpool.tile([2, 512], f32, tag="warmb")
                nc.gpsimd.memset(warmb[:], 1.0)
                warmr = warmb[:].bitcast(f32r)

                zf = mpool.tile([128, 16 * C], f32, tag="zf")
                nsplit = [(0, 512), (512, 512), (1024, QN - 1024)]

                def _xslice(cs, w, c):
                    # x cols for chunk-range [cs, cs+w), weight chunk c
                    s0, n = cs // 128, w // 128
                    v = xall[:].rearrange("p (s c x) -> p s c x", c=4, x=128)
                    return v[:, s0:s0 + n, c, :]

                # ---- finals: y=z-1, dr=1/(sum z - 9), out=z*dr-dr, out[C]=dr
                outq3 = mpool.tile([128, 4 * (C + 1)], f32, tag="outq3")

                def finals(m0, m1, outq):
                    # fully off-DVE: sum(y)-9 via Act accum_out (bias=-0.9
                    # per class column), 1/x via Pool normalize_recip.
                    n = m1 - m0
                    q = m0 // 4
                    drq = mpool.tile([128, n], f32, name=f"dr{m0}",
                                     tag=f"drq{n}", bufs=2)
                    junk = mpool.tile([128, C], f32, name=f"jk{m0}",
                                      tag="junk", bufs=2)
                    junk1 = mpool.tile([128, n], f32, name=f"jk1{m0}",
                                       tag="junk1", bufs=2)
                    for i in range(n):
                        nc.scalar.activation(
                            junk[:], zf[:, C * (m0 + i):C * (m0 + i + 1)],
                            Act.Identity, bias=nb9[:, 0:1],
                            accum_out=drq[:, i:i + 1])
                    for i in range(n):
                        nc.gpsimd.normalize_recip(
                            junk1[:, i:i + 1], drq[:, i:i + 1],
                            drq[:, i:i + 1])
                    ndr = mpool.tile([128, n], f32, name=f"ndr{m0}",
                                     tag=f"ndr{n}", bufs=2)
                    nc.gpsimd.tensor_scalar_mul(ndr[:], drq[:], -1.0)
                    for i in range(n):
                        s = m0 + i - 4 * q
                        nc.scalar.activation(
                            outq[:, (C + 1) * s:(C + 1) * s + C],
                            zf[:, C * (m0 + i):C * (m0 + i + 1)],
                            Act.Identity, scale=drq[:, i:i + 1],
                            bias=ndr[:, i:i + 1])
                    ovv = outq[:].rearrange("p (s k) -> p s k", k=C + 1)
                    nc.gpsimd.tensor_copy(ovv[:, m0 - 4 * q:m1 - 4 * q, C],
                                          drq[:])

                def finals_dve(m, sov, outq, ocol):
                    # minimal-latency all-DVE finals for the tail chunk,
                    # reading the scan output tile directly (no zf staging)
                    yv = sov[:, :, SEG - 1]                   # [128, C] strided
                    szq = mpool.tile([128, 1], f32, name=f"szd{m}", tag="szd")
                    nc.vector.tensor_reduce(szq[:], yv,
                                            axis=mybir.AxisListType.X,
                                            op=Alu.add)
                    nc.vector.tensor_scalar_add(szq[:], szq[:], -(C - 1.0))
                    nc.vector.reciprocal(outq[:, ocol + C:ocol + C + 1], szq[:])
                    dr = outq[:, ocol + C:ocol + C + 1]
                    nc.vector.tensor_scalar(
                        outq[:, ocol:ocol + C], yv,
                        scalar1=dr, scalar2=dr,
                        op0=Alu.mult, op1=Alu.subtract)

                def q_dma(q, outq):
                    nc.sync.dma_start(out_d[:, 44 * q:44 * (q + 1)], outq[:])

                # ---- chunk front: qs matmul -> (+1/3) -> scan ----
                so_map = {}

                def chunk_front(m, rho, joff):
                    on_pe = m in ONPE
                    qs = pq.tile([128, QN], f32, name=f"qs{m}", tag="qs")
                    so = spool.tile([128, QN], f32, name=f"so{m}", tag="so",
                                    bufs=6)
                    so_map[m] = so
                    sh = None
                    if not on_pe:
                        sh = spool.tile([128, QN], f32, name=f"sh{m}",
                                        tag="sh", bufs=3)
                    for (o, n) in nsplit:
                        nc.tensor.matmul(
                            qs[:, o:o + n],
                            rho[:, 128 * joff:128 * (joff + 1)],
                            ublk[:, o:o + n], start=True, stop=not on_pe)
                        if on_pe:
                            nc.tensor.matmul(
                                qs[:, o:o + n], ones2, crow2[:, o:o + n],
                                start=False, stop=True)
                    if on_pe:
                        d0 = qs
                    else:
                        nc.scalar.activation(sh[:], qs[:], Act.Copy,
                                             bias=1.0 / 3.0)
                        d0 = sh
                    nc.vector.tensor_tensor_scan(
                        so[:], d0[:], data1[:], initial=1.0,
                        op0=Alu.mult, op1=Alu.add)

                # ---- chunk post: z extract -> finals/DMA ----
                def fin_chunk(m, yv, outq, s):
                    # per-chunk finals fully off-DVE (yv: [128, C] z values)
                    drq = mpool.tile([128, 1], f32, name=f"dr{m}",
                                     tag="drq1", bufs=4)
                    junk = mpool.tile([128, C], f32, name=f"jk{m}",
                                      tag="junk", bufs=2)
                    junk1 = mpool.tile([128, 1], f32, name=f"jk1{m}",
                                       tag="junk1", bufs=2)
                    ndr = mpool.tile([128, 1], f32, name=f"ndr{m}",
                                     tag="ndr1", bufs=4)
                    nc.scalar.activation(junk[:], yv, Act.Identity,
                                         bias=nb9[:, 0:1],
                                         accum_out=drq[:])
                    nc.gpsimd.normalize_recip(junk1[:], drq[:], drq[:])
                    nc.gpsimd.tensor_scalar_mul(ndr[:], drq[:], -1.0)
                    nc.scalar.activation(
                        outq[:, (C + 1) * s:(C + 1) * s + C], yv,
                        Act.Identity, scale=drq[:, 0:1], bias=ndr[:, 0:1])
                    nc.gpsimd.tensor_copy(
                        outq[:, (C + 1) * s + C:(C + 1) * s + C + 1], drq[:])

                def chunk_post(m):
                    so = so_map.pop(m)
                    sov = so[:].rearrange("p (k s) -> p k s", s=SEG)
                    if m not in FINDVE:
                        nc.gpsimd.tensor_copy(
                            zf[:, C * m:C * (m + 1)], sov[:, :, SEG - 1])
                    # finals: batched per quarter; per-chunk on last quarter
                    if m in (3, 7, 11):
                        q = m // 4
                        oq = mpool.tile([128, 4 * (C + 1)], f32,
                                        name=f"outq{q}", tag="outq", bufs=2)
                        finals(4 * q, 4 * q + 4, oq)
                        q_dma(q, oq)
                    elif m == 14 and BATCH_Q3:
                        finals(12, 15, outq3)
                    elif m == 15:
                        if 15 in FINDVE:
                            finals_dve(15, sov, outq3, 33)
                        else:
                            fin_chunk(15, zf[:, C * 15:C * 16], outq3, 3)
                        nc.sync.dma_start(out_d[:, 132:176], outq3[:])

                # ---- per-slice pipeline (chunk stage lags TWO slices, and
                #      long-latency chain tails (Ln/rec/rho) are emitted
                #      after the older slice's chunk work: engine SEQs issue
                #      in order, so a waiting chain op ahead of a ready evac
                #      or qs matmul would stall the whole stream) ----
                pending = []          # (first_chunk, nchunks, rho_tile)
                flushed = []          # chunk ids with front emitted, post not

                def flush_front():
                    (m0, nch, rr) = pending.pop(0)
                    for j in range(nch):
                        chunk_front(m0 + j, rr, j)
                        flushed.append(m0 + j)

                def flush_posts():
                    while flushed:
                        chunk_post(flushed.pop(0))

                for si_idx, (cs, w) in enumerate(SLICES):
                    mode = RHO_MODE[si_idx]
                    if len(pending) == PIPE_LAG:
                        flush_front()
                    T = pst.tile([128, 512], f32, name=f"T{cs}", tag="T")
                    if si_idx == 0:
                        for _w in range(WARMUP):
                            nc.tensor.matmul(T[:, 0:512], wh[0:2, 0:128],
                                             wh[0:2, 0:512],
                                             start=True, stop=False)
                    for c in range(4):
                        nc.tensor.matmul(T[:, 0:w],
                                         wh[:, 128 * c:128 * (c + 1)],
                                         _xslice(cs, w, c),
                                         start=(c == 0), stop=False)
                    for c in range(4):
                        nc.tensor.matmul(
                            T[:, 0:w],
                            wh[:, 512 + 128 * c:512 + 128 * (c + 1)],
                            _xslice(cs, w, c),
                            start=False, stop=False)
                    nc.tensor.matmul(T[:, 0:w], ones2, cb2[:, cs:cs + w],
                                     start=False, stop=True)

                    si = mpool.tile([128, 512], f32, name=f"si{cs}", tag="si",
                                    bufs=SI_BUFS)
                    nc.scalar.activation(si[:, 0:w], T[:, 0:w], Act.Exp,
                                         bias=crowb[:, 0:1])
                    amax = mpool.tile([128, 512], f32, name=f"am{cs}",
                                      tag="amax", bufs=2)
                    nc.gpsimd.partition_all_reduce(
                        amax[:, 0:w], si[:, 0:w], channels=128,
                        reduce_op=bass_isa.ReduceOp.max)
                    rho = mpool.tile([128, 512], f16, name=f"rho{cs}",
                                     tag="rho", bufs=RHO_BUFS)
                    if mode in ("dve", "mix", "mixp"):
                        dent = mpool.tile([128, 512], f32, name=f"dent{cs}",
                                          tag="dentv", bufs=2)
                        if mode == "mixp":
                            d0p = mpool.tile([128, 512], f32, name=f"d0{cs}",
                                             tag="d0p", bufs=2)
                            nc.gpsimd.tensor_sub(d0p[:, 0:w], amax[:, 0:w],
                                                 si[:, 0:w])
                            nc.gpsimd.tensor_scalar_add(dent[:, 0:w],
                                                        d0p[:, 0:w], 1e-4)
                        else:
                            nc.vector.scalar_tensor_tensor(
                                dent[:, 0:w], amax[:, 0:w], 1e-4, si[:, 0:w],
                                op0=Alu.add, op1=Alu.subtract)
                        rec = mpool.tile([128, 512], f32, name=f"rec{cs}",
                                         tag="rec", bufs=2)
                        nc.vector.reciprocal_approx_fast(rec[:, 0:w],
                                                         dent[:, 0:w])
                        if mode == "dve":
                            nc.vector.tensor_mul(rho[:, 0:w], si[:, 0:w],
                                                 rec[:, 0:w])
                        else:
                            nc.gpsimd.tensor_mul(rho[:, 0:w], si[:, 0:w],
                                                 rec[:, 0:w])
                    else:
                        d0p = mpool.tile([128, 512], f32, name=f"d0{cs}",
                                         tag="d0p", bufs=2)
                        nc.gpsimd.tensor_sub(d0p[:, 0:w], amax[:, 0:w],
                                             si[:, 0:w])
                        dent = mpool.tile([128, 512], f32, name=f"dent{cs}",
                                          tag="dentp", bufs=2)
                        nc.gpsimd.tensor_scalar_add(dent[:, 0:w], d0p[:, 0:w],
                                                    1e-4)
                        ldt = mpool.tile([128, 512], f32, name=f"ld{cs}",
                                         tag="ldt", bufs=2)
                        nc.scalar.activation(ldt[:, 0:w], dent[:, 0:w], Act.Ln)
                        rec = mpool.tile([128, 512], f32, name=f"rec{cs}",
                                         tag="rec", bufs=2)
                        nc.scalar.activation(rec[:, 0:w], ldt[:, 0:w], Act.Exp,
                                             scale=-1.0)
                        nc.gpsimd.tensor_mul(rho[:, 0:w], si[:, 0:w],
                                             rec[:, 0:w])
                    flush_posts()
                    pending.append((cs // 128, w // 128, rho))
                while pending:
                    flush_front()
                    flush_posts()

    nc.compile()
    # The act-table insertion pass picks tables greedily per function (Exp ->
    # exp_and_others id 0, Ln -> natural_log id 5) and thrashes 1283ns loads
    # between them.  Table 6 (natural_log_exp_and_others) contains Exp, Ln,
    # Copy and Identity together: re-point every load at it and drop the
    # now-redundant swaps (only ones carrying no semaphore info).
    for blk in nc.main_func.blocks:
        keep = []
        loaded = False
        for inst in blk.instructions:
            if isinstance(inst, mybir.InstLoadActFuncSet) \
                    and inst.act_func_set_id in (0, 5, 6):
                si = getattr(inst, "sync_info", None)
                empty = si is None or (not si.on_wait and not si.on_update)
                if loaded and empty:
                    continue
                inst.act_func_set_id = 6
                loaded = True
            keep.append(inst)
        blk.instructions[:] = keep
    return nc


def _f32r_round(v):
    # float32r = RNE to 11 explicit mantissa bits (HW-verified).
    u = np.asarray(v, np.float32).view(np.uint32).astype(np.uint64)
    drop = 12
    half = np.uint64(1 << (drop - 1))
    odd = (u >> np.uint64(drop)) & np.uint64(1)
    u2 = (u + half - np.uint64(1) + odd) & np.uint64(~((1 << drop) - 1) & 0xFFFFFFFF)
    return u2.astype(np.uint32).view(np.float32)


def _host_prep(x, w, eta, xi, beta):
    """Host-side: shard/layout x, build tiny replicated param matrices."""
    x = np.asarray(x, np.float32)
    w = np.asarray(w, np.float32)
    eta = np.asarray(eta, np.float32).reshape(-1)
    xi = np.asarray(xi, np.float32).reshape(-1)
    beta = np.asarray(beta, np.float32)

    gamma = (eta.astype(np.float64)) ** 2                # [P]
    if np.ptp(gamma) != 0.0:
        raise NotImplementedError(
            "kernel assumes per-prototype-constant gamma (eta); the shipped "
            "problem uses eta = full(0.1)")
    g0 = float(gamma[0])
    alpha = 1.0 / (1.0 + np.exp(-xi.astype(np.float64)))
    wsq = (w.astype(np.float64) ** 2).sum(-1)            # [P]

    wt2g = 2.0 * gamma[None, :] * w.T.astype(np.float64)   # [F,P] f64
    whb = wt2g.astype(np.float16)                        # [F, P] fp16 hi
    wrb = (wt2g - whb.astype(np.float64)).astype(np.float16)  # fp16 residual
    whp = np.zeros((128, 1024), np.float16)
    for c in range(4):
        whp[:, 128 * c:128 * (c + 1)] = whb[128 * c:128 * (c + 1), :]
        whp[:, 512 + 128 * c:512 + 128 * (c + 1)] = wrb[128 * c:128 * (c + 1), :]

    crow_bias = (np.log(alpha) - gamma * wsq).astype(np.float32)[:, None]  # [P,1]

    b2 = beta.astype(np.float64) ** 2
    u = b2 / b2.sum(0, keepdims=True)                    # [C,P]
    uh = u / 3.0
    third_a = float(_f32r_round(np.float32(1.0 / 3.0)))
    third_b = np.float32(1.0 / 3.0 - third_a)
    ublk = np.zeros((P, QN), np.float16)
    crow2ab = np.zeros((2, QN), np.float32)
    for k in range(C):
        base = k * SEG
        crow2ab[0, base + 1:base + SEG] = third_a
        crow2ab[1, base + 1:base + SEG] = third_b
        for t in range(P):
            v = uh[k, t] * (3.0 if t == 0 else 1.0)
            ublk[t, base + 1 + t] = np.float16(v)

    shards = x.reshape(NCORES, BL, F)
    in_maps = []
    for i in range(NCORES):
        xs = shards[i]                                    # [BL, F] f32
        xt = np.ascontiguousarray(xs.T)                   # [F, BL]
        xh = xt.astype(np.float16)
        # slice-contiguous: block (slice s of 16, chunk c) at col s*512+c*128
        xqp = np.zeros((128, 8192), np.float16)
        for s in range(16):
            for c in range(4):
                xqp[:, 512 * s + 128 * c:512 * s + 128 * (c + 1)] = \
                    xh[128 * c:128 * (c + 1), 128 * s:128 * (s + 1)]
        # -g|x|^2 per batch row, f64 -> f32r a + f32r b residual rows
        cb = -(g0 * (xs.astype(np.float64) ** 2).sum(-1))          # [BL]
        cba = _f32r_round(cb.astype(np.float32))
        cbb = _f32r_round((cb - cba.astype(np.float64)).astype(np.float32))
        cb2 = np.stack([cba, cbb], axis=0)                         # [2, BL]
        in_maps.append({
            "xq": xqp, "whp": whp, "cb2": cb2, "crowb": crow_bias,
            "ublk": ublk, "crow2ab": crow2ab,
        })
    return in_maps


def _run(in_maps, trace=False):
    from concourse.bass_utils import run_bass_kernel_spmd

    if "nc" not in _PROG:
        _PROG["nc"] = _build_program()
    nc = _PROG["nc"]
    res = run_bass_kernel_spmd(
        nc, in_maps, core_ids=list(range(NCORES)), trace=trace)
    outs = []
    for i in range(NCORES):
        o = np.asarray(res.results[i]["out"])          # [128, 176]
        outs.append(o.reshape(128, 16, C + 1).transpose(1, 0, 2).reshape(BL, C + 1))
    full = np.concatenate(outs, axis=0).astype(np.float32)
    return full, res


def kernel(x, w, eta, xi, beta):
    in_maps = _host_prep(x, w, eta, xi, beta)
    full, _ = _run(in_maps, trace=False)
    return full


# revision 39
# speedup vs baseline: 1.0063x; 1.0063x over previous
"""Dempster-Shafer evidential module on 8 Trainium2 cores.

Math: the reference's per-step Dempster normalization cancels, so the scan
collapses to an affine recurrence per (batch b, class k):

    z_t = shat[b,t,k]*z_{t-1} + 2/3,   z after prototype 0 = 1 + u[k,0]*rho[b,0]
    shat = 1/3 + (u/3)*rho,  rho = si/(maxsi + 1e-4 - si),  si = exp(T)
    T[p,b] = 2g x.w_p - g|w_p|^2 + ln a - g|x|^2
    y = z_T - 1;  out[b,k] = y/(sum_k y + 1);  out[b,C] = 1/(sum_k y + 1)

v2 structure (43.4us -> target ~32us; TimelineSim cost model):
  - DVE reduced to (nearly) scans only: the reciprocal chain moved off DVE
    via rec = Exp(-Ln(dent)) on Act (error ~1e-6, below the f32r rho
    quantization).  dent on Pool (2 ops) for slices 2+; slices 0-1 keep the
    short all-DVE chain (dent STT -> rec approx -> rho mult) since DVE is
    idle pre-stream anyway.
  - ONPE chunks: +1/3 comes from ONE K=2 matmul pass (crow2 as [2, QN]
    f32r a+b pair rows); the scan reads qs straight from PSUM (+65ns/scan).
    Remaining chunks use the Act evac as before.  Balances Act vs PE.
  - finals: reduce on Pool, tiny add/reciprocal on DVE, out-writes on Act
    (Copy scale=drq bias=-drq); last chunk (m=15) runs a minimal all-DVE
    chain reading the scan output directly to shorten the tail.
  - DMA: crowb/x0/wh/cb2/crow2 early on HWDGE, ublk split into 3 pieces,
    xs1 via the gpsimd SWDGE queue in parallel.
"""

import numpy as np

B, F, P, C = 16384, 512, 128, 10
NCORES = 8
BL = B // NCORES          # 2048 rows per core
SEG = P + 1               # 129 columns per class segment
QN = C * SEG              # 1290 scan columns
OUTW = 16 * (C + 1)       # 176 packed output columns

# batch-column slices (start, ncols); chunk m = col/128, 16 chunks total
SLICES = [(0, 128), (128, 128), (256, 256), (512, 512), (1024, 512),
          (1536, 512)]
ONPE = (0, 2, 4, 6, 8, 10, 12, 14)   # chunks: +1/3 via K=2 crow2 matmul,
                                      # scan reads PSUM (no Act evac)
# per-slice rho-chain mode: "dve" = dent/rec/rho all DVE (pre-stream window)
# "lnexp" = dent on Pool, rec = Exp(-Ln(dent)) on Act, rho mult on Pool
RHO_MODE = ("dve", "dve", "mix", "mix", "mix", "mix")
DMA_ORDER = ["wh", "xs0", "cb2", "crowb", "crow2p", "ublkAp", "xs1",
             "xs23", "ublkB", "ublkC", "x47", "x811", "x1215"]
FINDVE = (15,)            # chunks with the minimal all-DVE finals tail
BATCH_Q3 = True
LINEARIZE = False
SCAN_BUFS = 3
POOL_MODE = "stack"
SI_BUFS = 4
RHO_BUFS = 5
PIPE_LAG = 3
FIN_REDUCE_POOL = False   # gpsimd tensor_reduce is partition-axis only
WARMUP = 2

_PROG = {}
REPS = 1


def _build_program():
    import concourse.bacc as bacc
    import concourse.bass as bass
    import concourse.tile as tile
    from concourse import bass_isa, mybir

    f32 = mybir.dt.float32
    f16 = mybir.dt.float16
    f32r = mybir.dt.float32r
    Alu = mybir.AluOpType
    Act = mybir.ActivationFunctionType

    nc = bacc.Bacc("TRN2", target_bir_lowering=False, debug=False)

    # x slice-contiguous: 128-col block (slice s, chunk c) at col s*512+c*128
    xq_d = nc.dram_tensor("xq", [128, 8192], f16, kind="ExternalInput").ap()
    wh_d = nc.dram_tensor("whp", [128, 1024], f16, kind="ExternalInput").ap()
    cb2_d = nc.dram_tensor("cb2", [2, BL], f32r, kind="ExternalInput").ap()
    crowb_d = nc.dram_tensor("crowb", [128, 1], f32, kind="ExternalInput").ap()
    ublk_d = nc.dram_tensor("ublk", [P, QN], f16, kind="ExternalInput").ap()
    c2ab_d = nc.dram_tensor("crow2ab", [2, QN], f32r, kind="ExternalInput").ap()
    out_d = nc.dram_tensor("out", [128, OUTW], f32, kind="ExternalOutput").ap()

    with tile.TileContext(nc, linearize=LINEARIZE,
                          pool_alloc_mode=POOL_MODE) as tc:
        for _rep in range(REPS):
            with (
                tc.tile_pool(name="const", bufs=1) as cpool,
                tc.tile_pool(name="xin", bufs=1) as xpool,
                tc.tile_pool(name="mid", bufs=1) as mpool,
                tc.tile_pool(name="scan", bufs=SCAN_BUFS) as spool,
                tc.tile_pool(name="pst", bufs=2, space=bass.MemorySpace.PSUM) as pst,
                tc.tile_pool(name="pq", bufs=2, space=bass.MemorySpace.PSUM) as pq,
            ):
                # ---- input DMAs (HWDGE serializes at 625ns/descriptor; DMA
                #      transfers serialize at ~360B/ns: order = startup path) ----
                xall = xpool.tile([128, 8192], f16, tag="xall")
                wh = cpool.tile([128, 1024], f16, tag="wh")
                cb2 = cpool.tile([2, BL], f32r, tag="cb2")
                crowb = cpool.tile([128, 1], f32, tag="crowb")
                ublk = cpool.tile([P, QN], f16, tag="ublk")
                crow2 = cpool.tile([2, QN], f32r, tag="crow2")

                dmas = {
                    "xs0": lambda: nc.sync.dma_start(xall[:, 0:512], xq_d[:, 0:512]),
                    "xs0p": lambda: nc.gpsimd.dma_start(xall[:, 0:512], xq_d[:, 0:512]),
                    "wh": lambda: nc.sync.dma_start(wh[:], wh_d[:]),
                    "whp": lambda: nc.gpsimd.dma_start(wh[:], wh_d[:]),
                    "cb2": lambda: nc.sync.dma_start(cb2[:], cb2_d[:]),
                    "cb2p": lambda: nc.gpsimd.dma_start(cb2[:], cb2_d[:]),
                    "crowbp": lambda: nc.gpsimd.dma_start(crowb[:], crowb_d[:]),
                    "crow2p": lambda: nc.gpsimd.dma_start(crow2[:], c2ab_d[:]),
                    "crowb": lambda: nc.sync.dma_start(crowb[:], crowb_d[:]),
                    "crow2": lambda: nc.sync.dma_start(crow2[:], c2ab_d[:]),
                    "ublk": lambda: nc.sync.dma_start(ublk[:], ublk_d[:]),
                    "ublkp": lambda: nc.gpsimd.dma_start(ublk[:], ublk_d[:]),
                    "ublkA": lambda: nc.sync.dma_start(ublk[:, 0:512], ublk_d[:, 0:512]),
                    "ublkAp": lambda: nc.gpsimd.dma_start(ublk[:, 0:512], ublk_d[:, 0:512]),
                    "ublkB": lambda: nc.sync.dma_start(ublk[:, 512:1024], ublk_d[:, 512:1024]),
                    "ublkC": lambda: nc.sync.dma_start(ublk[:, 1024:QN], ublk_d[:, 1024:QN]),
                    "xs1": lambda: nc.sync.dma_start(xall[:, 512:1024], xq_d[:, 512:1024]),
                    "xs1p": lambda: nc.gpsimd.dma_start(xall[:, 512:1024], xq_d[:, 512:1024]),
                    "xs23": lambda: nc.sync.dma_start(xall[:, 1024:2048], xq_d[:, 1024:2048]),
                    "x47": lambda: nc.sync.dma_start(xall[:, 2048:4096], xq_d[:, 2048:4096]),
                    "x811": lambda: nc.sync.dma_start(xall[:, 4096:6144], xq_d[:, 4096:6144]),
                    "x1215": lambda: nc.sync.dma_start(xall[:, 6144:8192], xq_d[:, 6144:8192]),
                }
                # ---- device-built constants (memsets precede the Pool
                #      SWDGE issues so nothing queues behind them) ----
                ones2f = cpool.tile([2, 128], f32, tag="ones2")
                nc.gpsimd.memset(ones2f[:], 1.0)
                ones2 = ones2f[:].bitcast(f32r)
                data1 = cpool.tile([128, QN], f32, tag="data1")
                nc.gpsimd.memset(data1[:], 2.0 / 3.0)
                d1v = data1[:].rearrange("p (k s) -> p k s", s=SEG)
                nc.gpsimd.memset(d1v[:, :, 0], 1.0)

                for k in DMA_ORDER:
                    dmas[k]()
                actw = cpool.tile([2, 1], f32, tag="actw")
                nc.scalar.activation(actw[:], ones2f[:, 0:1], Act.Exp)

                nb9 = cpool.tile([128, 1], f32, tag="nb9")
                nc.gpsimd.memset(nb9[:], -(C - 1.0) / C)
                warmb = cpool.tile([2, 512], f32, tag="warmb")
                nc.gpsimd.memset(warmb[:], 1.0)
                warmr = warmb[:].bitcast(f32r)

                zf = mpool.tile([128, 16 * C], f32, tag="zf")
                nsplit = [(0, 512), (512, 512), (1024, QN - 1024)]

                def _xslice(cs, w, c):
                    # x cols for chunk-range [cs, cs+w), weight chunk c
                    s0, n = cs // 128, w // 128
                    v = xall[:].rearrange("p (s c x) -> p s c x", c=4, x=128)
                    return v[:, s0:s0 + n, c, :]

                # ---- finals: y=z-1, dr=1/(sum z - 9), out=z*dr-dr, out[C]=dr
                outq3 = mpool.tile([128, 4 * (C + 1)], f32, tag="outq3")

                def finals(m0, m1, outq):
                    # fully off-DVE: sum(y)-9 via Act accum_out (bias=-0.9
                    # per class column), 1/x via Pool normalize_recip.
                    n = m1 - m0
                    q = m0 // 4
                    drq = mpool.tile([128, n], f32, name=f"dr{m0}",
                                     tag=f"drq{n}", bufs=2)
                    junk = mpool.tile([128, C], f32, name=f"jk{m0}",
                                      tag="junk", bufs=2)
                    junk1 = mpool.tile([128, n], f32, name=f"jk1{m0}",
                                       tag="junk1", bufs=2)
                    for i in range(n):
                        nc.scalar.activation(
                            junk[:], zf[:, C * (m0 + i):C * (m0 + i + 1)],
                            Act.Identity, bias=nb9[:, 0:1],
                            accum_out=drq[:, i:i + 1])
                    for i in range(n):
                        nc.gpsimd.normalize_recip(
                            junk1[:, i:i + 1], drq[:, i:i + 1],
                            drq[:, i:i + 1])
                    ndr = mpool.tile([128, n], f32, name=f"ndr{m0}",
                                     tag=f"ndr{n}", bufs=2)
                    nc.gpsimd.tensor_scalar_mul(ndr[:], drq[:], -1.0)
                    for i in range(n):
                        s = m0 + i - 4 * q
                        nc.scalar.activation(
                            outq[:, (C + 1) * s:(C + 1) * s + C],
                            zf[:, C * (m0 + i):C * (m0 + i + 1)],
                            Act.Identity, scale=drq[:, i:i + 1],
                            bias=ndr[:, i:i + 1])
                    ovv = outq[:].rearrange("p (s k) -> p s k", k=C + 1)
                    nc.gpsimd.tensor_copy(ovv[:, m0 - 4 * q:m1 - 4 * q, C],
                                          drq[:])

                def finals_dve(m, sov, outq, ocol):
                    # minimal-latency all-DVE finals for the tail chunk,
                    # reading the scan output tile directly (no zf staging)
                    yv = sov[:, :, SEG - 1]                   # [128, C] strided
                    szq = mpool.tile([128, 1], f32, name=f"szd{m}", tag="szd")
                    nc.vector.tensor_reduce(szq[:], yv,
                                            axis=mybir.AxisListType.X,
                                            op=Alu.add)
                    nc.vector.tensor_scalar_add(szq[:], szq[:], -(C - 1.0))
                    nc.vector.reciprocal(outq[:, ocol + C:ocol + C + 1], szq[:])
                    dr = outq[:, ocol + C:ocol + C + 1]
                    nc.vector.tensor_scalar(
                        outq[:, ocol:ocol + C], yv,
                        scalar1=dr, scalar2=dr,
                        op0=Alu.mult, op1=Alu.subtract)

                def q_dma(q, outq):
                    nc.sync.dma_start(out_d[:, 44 * q:44 * (q + 1)], outq[:])

                # ---- chunk front: qs matmul -> (+1/3) -> scan ----
                so_map = {}

                def chunk_front(m, rho, joff):
                    on_pe = m in ONPE
                    qs = pq.tile([128, QN], f32, name=f"qs{m}", tag="qs")
                    so = spool.tile([128, QN], f32, name=f"so{m}", tag="so",
                                    bufs=6)
                    so_map[m] = so
                    sh = None
                    if not on_pe:
                        sh = spool.tile([128, QN], f32, name=f"sh{m}",
                                        tag="sh", bufs=3)
                    for (o, n) in nsplit:
                        nc.tensor.matmul(
                            qs[:, o:o + n],
                            rho[:, 128 * joff:128 * (joff + 1)],
                            ublk[:, o:o + n], start=True, stop=not on_pe)
                        if on_pe:
                            nc.tensor.matmul(
                                qs[:, o:o + n], ones2, crow2[:, o:o + n],
                                start=False, stop=True)
                    if on_pe:
                        d0 = qs
                    else:
                        nc.scalar.activation(sh[:], qs[:], Act.Copy,
                                             bias=1.0 / 3.0)
                        d0 = sh
                    nc.vector.tensor_tensor_scan(
                        so[:], d0[:], data1[:], initial=1.0,
                        op0=Alu.mult, op1=Alu.add)

                # ---- chunk post: z extract -> finals/DMA ----
                def fin_chunk(m, yv, outq, s):
                    # per-chunk finals fully off-DVE (yv: [128, C] z values)
                    drq = mpool.tile([128, 1], f32, name=f"dr{m}",
                                     tag="drq1", bufs=4)
                    junk = mpool.tile([128, C], f32, name=f"jk{m}",
                                      tag="junk", bufs=2)
                    junk1 = mpool.tile([128, 1], f32, name=f"jk1{m}",
                                       tag="junk1", bufs=2)
                    ndr = mpool.tile([128, 1], f32, name=f"ndr{m}",
                                     tag="ndr1", bufs=4)
                    nc.scalar.activation(junk[:], yv, Act.Identity,
                                         bias=nb9[:, 0:1],
                                         accum_out=drq[:])
                    nc.gpsimd.normalize_recip(junk1[:], drq[:], drq[:])
                    nc.gpsimd.tensor_scalar_mul(ndr[:], drq[:], -1.0)
                    nc.scalar.activation(
                        outq[:, (C + 1) * s:(C + 1) * s + C], yv,
                        Act.Identity, scale=drq[:, 0:1], bias=ndr[:, 0:1])
                    nc.gpsimd.tensor_copy(
                        outq[:, (C + 1) * s + C:(C + 1) * s + C + 1], drq[:])

                def chunk_post(m):
                    so = so_map.pop(m)
                    sov = so[:].rearrange("p (k s) -> p k s", s=SEG)
                    if m not in FINDVE:
                        nc.gpsimd.tensor_copy(
                            zf[:, C * m:C * (m + 1)], sov[:, :, SEG - 1])
                    # finals: batched per quarter; per-chunk on last quarter
                    if m in (3, 7, 11):
                        q = m // 4
                        oq = mpool.tile([128, 4 * (C + 1)], f32,
                                        name=f"outq{q}", tag="outq", bufs=2)
                        finals(4 * q, 4 * q + 4, oq)
                        q_dma(q, oq)
                    elif m == 14 and BATCH_Q3:
                        finals(12, 15, outq3)
                    elif m == 15:
                        if 15 in FINDVE:
                            finals_dve(15, sov, outq3, 33)
                        else:
                            fin_chunk(15, zf[:, C * 15:C * 16], outq3, 3)
                        nc.sync.dma_start(out_d[:, 132:176], outq3[:])

                # ---- per-slice pipeline (chunk stage lags TWO slices, and
                #      long-latency chain tails (Ln/rec/rho) are emitted
                #      after the older slice's chunk work: engine SEQs issue
                #      in order, so a waiting chain op ahead of a ready evac
                #      or qs matmul would stall the whole stream) ----
                pending = []          # (first_chunk, nchunks, rho_tile)
                flushed = []          # chunk ids with front emitted, post not

                def flush_front():
                    (m0, nch, rr) = pending.pop(0)
                    for j in range(nch):
                        chunk_front(m0 + j, rr, j)
                        flushed.append(m0 + j)

                def flush_posts():
                    while flushed:
                        chunk_post(flushed.pop(0))

                for si_idx, (cs, w) in enumerate(SLICES):
                    mode = RHO_MODE[si_idx]
                    if len(pending) == PIPE_LAG:
                        flush_front()
                    T = pst.tile([128, 512], f32, name=f"T{cs}", tag="T")
                    if si_idx == 0:
                        for _w in range(WARMUP):
                            nc.tensor.matmul(T[:, 0:512], wh[0:2, 0:128],
                                             wh[0:2, 0:512],
                                             start=True, stop=False)
                    for c in range(4):
                        nc.tensor.matmul(T[:, 0:w],
                                         wh[:, 128 * c:128 * (c + 1)],
                                         _xslice(cs, w, c),
                                         start=(c == 0), stop=False)
                    for c in range(4):
                        nc.tensor.matmul(
                            T[:, 0:w],
                            wh[:, 512 + 128 * c:512 + 128 * (c + 1)],
                            _xslice(cs, w, c),
                            start=False, stop=False)
                    nc.tensor.matmul(T[:, 0:w], ones2, cb2[:, cs:cs + w],
                                     start=False, stop=True)

                    si = mpool.tile([128, 512], f32, name=f"si{cs}", tag="si",
                                    bufs=SI_BUFS)
                    nc.scalar.activation(si[:, 0:w], T[:, 0:w], Act.Exp,
                                         bias=crowb[:, 0:1])
                    amax = mpool.tile([128, 512], f32, name=f"am{cs}",
                                      tag="amax", bufs=2)
                    nc.gpsimd.partition_all_reduce(
                        amax[:, 0:w], si[:, 0:w], channels=128,
                        reduce_op=bass_isa.ReduceOp.max)
                    rho = mpool.tile([128, 512], f16, name=f"rho{cs}",
                                     tag="rho", bufs=RHO_BUFS)
                    if mode in ("dve", "mix", "mixp"):
                        dent = mpool.tile([128, 512], f32, name=f"dent{cs}",
                                          tag="dentv", bufs=2)
                        if mode == "mixp":
                            d0p = mpool.tile([128, 512], f32, name=f"d0{cs}",
                                             tag="d0p", bufs=2)
                            nc.gpsimd.tensor_sub(d0p[:, 0:w], amax[:, 0:w],
                                                 si[:, 0:w])
                            nc.gpsimd.tensor_scalar_add(dent[:, 0:w],
                                                        d0p[:, 0:w], 1e-4)
                        else:
                            nc.vector.scalar_tensor_tensor(
                                dent[:, 0:w], amax[:, 0:w], 1e-4, si[:, 0:w],
                                op0=Alu.add, op1=Alu.subtract)
                        rec = mpool.tile([128, 512], f32, name=f"rec{cs}",
                                         tag="rec", bufs=2)
                        nc.vector.reciprocal_approx_fast(rec[:, 0:w],
                                                         dent[:, 0:w])
                        if mode == "dve":
                            nc.vector.tensor_mul(rho[:, 0:w], si[:, 0:w],
                                                 rec[:, 0:w])
                        else:
                            nc.gpsimd.tensor_mul(rho[:, 0:w], si[:, 0:w],
                                                 rec[:, 0:w])
                    else:
                        d0p = mpool.tile([128, 512], f32, name=f"d0{cs}",
                                         tag="d0p", bufs=2)
                        nc.gpsimd.tensor_sub(d0p[:, 0:w], amax[:, 0:w],
                                             si[:, 0:w])
                        dent = mpool.tile([128, 512], f32, name=f"dent{cs}",
                                          tag="dentp", bufs=2)
                        nc.gpsimd.tensor_scalar_add(dent[:, 0:w], d0p[:, 0:w],
                                                    1e-4)
                        ldt = mpool.tile([128, 512], f32, name=f"ld{cs}",
                                         tag="ldt", bufs=2)
                        nc.scalar.activation(ldt[:, 0:w], dent[:, 0:w], Act.Ln)
                        rec = mpool.tile([128, 512], f32, name=f"rec{cs}",
                                         tag="rec", bufs=2)
                        nc.scalar.activation(rec[:, 0:w], ldt[:, 0:w], Act.Exp,
                                             scale=-1.0)
                        nc.gpsimd.tensor_mul(rho[:, 0:w], si[:, 0:w],
                                             rec[:, 0:w])
                    flush_posts()
                    pending.append((cs // 128, w // 128, rho))
                while pending:
                    flush_front()
                    flush_posts()

    nc.compile()
    # The act-table insertion pass picks tables greedily per function (Exp ->
    # exp_and_others id 0, Ln -> natural_log id 5) and thrashes 1283ns loads
    # between them.  Table 6 (natural_log_exp_and_others) contains Exp, Ln,
    # Copy and Identity together: re-point every load at it and drop the
    # now-redundant swaps (only ones carrying no semaphore info).
    for blk in nc.main_func.blocks:
        keep = []
        loaded = False
        for inst in blk.instructions:
            if isinstance(inst, mybir.InstLoadActFuncSet) \
                    and inst.act_func_set_id in (0, 5, 6):
                si = getattr(inst, "sync_info", None)
                empty = si is None or (not si.on_wait and not si.on_update)
                if loaded and empty:
                    continue
                inst.act_func_set_id = 6
                loaded = True
            keep.append(inst)
        blk.instructions[:] = keep
    return nc


def _f32r_round(v):
    # float32r = RNE to 11 explicit mantissa bits (HW-verified).
    u = np.asarray(v, np.float32).view(np.uint32).astype(np.uint64)
    drop = 12
    half = np.uint64(1 << (drop - 1))
    odd = (u >> np.uint64(drop)) & np.uint64(1)
    u2 = (u + half - np.uint64(1) + odd) & np.uint64(~((1 << drop) - 1) & 0xFFFFFFFF)
    return u2.astype(np.uint32).view(np.float32)


def _host_prep(x, w, eta, xi, beta):
    """Host-side: shard/layout x, build tiny replicated param matrices."""
    x = np.asarray(x, np.float32)
    w = np.asarray(w, np.float32)
    eta = np.asarray(eta, np.float32).reshape(-1)
    xi = np.asarray(xi, np.float32).reshape(-1)
    beta = np.asarray(beta, np.float32)

    gamma = (eta.astype(np.float64)) ** 2                # [P]
    if np.ptp(gamma) != 0.0:
        raise NotImplementedError(
            "kernel assumes per-prototype-constant gamma (eta); the shipped "
            "problem uses eta = full(0.1)")
    g0 = float(gamma[0])
    alpha = 1.0 / (1.0 + np.exp(-xi.astype(np.float64)))
    wsq = (w.astype(np.float64) ** 2).sum(-1)            # [P]

    wt2g = 2.0 * gamma[None, :] * w.T.astype(np.float64)   # [F,P] f64
    whb = wt2g.astype(np.float16)                        # [F, P] fp16 hi
    wrb = (wt2g - whb.astype(np.float64)).astype(np.float16)  # fp16 residual
    whp = np.zeros((128, 1024), np.float16)
    for c in range(4):
        whp[:, 128 * c:128 * (c + 1)] = whb[128 * c:128 * (c + 1), :]
        whp[:, 512 + 128 * c:512 + 128 * (c + 1)] = wrb[128 * c:128 * (c + 1), :]

    crow_bias = (np.log(alpha) - gamma * wsq).astype(np.float32)[:, None]  # [P,1]

    b2 = beta.astype(np.float64) ** 2
    u = b2 / b2.sum(0, keepdims=True)                    # [C,P]
    uh = u / 3.0
    third_a = float(_f32r_round(np.float32(1.0 / 3.0)))
    third_b = np.float32(1.0 / 3.0 - third_a)
    ublk = np.zeros((P, QN), np.float16)
    crow2ab = np.zeros((2, QN), np.float32)
    for k in range(C):
        base = k * SEG
        crow2ab[0, base + 1:base + SEG] = third_a
        crow2ab[1, base + 1:base + SEG] = third_b
        for t in range(P):
            v = uh[k, t] * (3.0 if t == 0 else 1.0)
            ublk[t, base + 1 + t] = np.float16(v)

    shards = x.reshape(NCORES, BL, F)
    in_maps = []
    for i in range(NCORES):
        xs = shards[i]                                    # [BL, F] f32
        xt = np.ascontiguousarray(xs.T)                   # [F, BL]
        xh = xt.astype(np.float16)
        # slice-contiguous: block (slice s of 16, chunk c) at col s*512+c*128
        xqp = np.zeros((128, 8192), np.float16)
        for s in range(16):
            for c in range(4):
                xqp[:, 512 * s + 128 * c:512 * s + 128 * (c + 1)] = \
                    xh[128 * c:128 * (c + 1), 128 * s:128 * (s + 1)]
        # -g|x|^2 per batch row, f64 -> f32r a + f32r b residual rows
        cb = -(g0 * (xs.astype(np.float64) ** 2).sum(-1))          # [BL]
        cba = _f32r_round(cb.astype(np.float32))
        cbb = _f32r_round((cb - cba.astype(np.float64)).astype(np.float32))
        cb2 = np.stack([cba, cbb], axis=0)                         # [2, BL]
        in_maps.append({
            "xq": xqp, "whp": whp, "cb2": cb2, "crowb": crow_bias,
            "ublk": ublk, "crow2ab": crow2ab,
        })
    return in_maps


def _run(in_maps, trace=False):
    from concourse.bass_utils import run_bass_kernel_spmd

    if "nc" not in _PROG:
        _PROG["nc"] = _build_program()
    nc = _PROG["nc"]
    res = run_bass_kernel_spmd(
        nc, in_maps, core_ids=list(range(NCORES)), trace=trace)
    outs = []
    for i in range(NCORES):
        o = np.asarray(res.results[i]["out"])          # [128, 176]
        outs.append(o.reshape(128, 16, C + 1).transpose(1, 0, 2).reshape(BL, C + 1))
    full = np.concatenate(outs, axis=0).astype(np.float32)
    return full, res


def kernel(x, w, eta, xi, beta):
    in_maps = _host_prep(x, w, eta, xi, beta)
    full, _ = _run(in_maps, trace=False)
    return full


# revision 40
# speedup vs baseline: 1.0131x; 1.0068x over previous
"""Dempster-Shafer evidential module on 8 Trainium2 cores.

Math: the reference's per-step Dempster normalization cancels, so the scan
collapses to an affine recurrence per (batch b, class k):

    z_t = shat[b,t,k]*z_{t-1} + 2/3,   z after prototype 0 = 1 + u[k,0]*rho[b,0]
    shat = 1/3 + (u/3)*rho,  rho = si/(maxsi + 1e-4 - si),  si = exp(T)
    T[p,b] = 2g x.w_p - g|w_p|^2 + ln a - g|x|^2
    y = z_T - 1;  out[b,k] = y/(sum_k y + 1);  out[b,C] = 1/(sum_k y + 1)

v2 structure (43.4us -> target ~32us; TimelineSim cost model):
  - DVE reduced to (nearly) scans only: the reciprocal chain moved off DVE
    via rec = Exp(-Ln(dent)) on Act (error ~1e-6, below the f32r rho
    quantization).  dent on Pool (2 ops) for slices 2+; slices 0-1 keep the
    short all-DVE chain (dent STT -> rec approx -> rho mult) since DVE is
    idle pre-stream anyway.
  - ONPE chunks: +1/3 comes from ONE K=2 matmul pass (crow2 as [2, QN]
    f32r a+b pair rows); the scan reads qs straight from PSUM (+65ns/scan).
    Remaining chunks use the Act evac as before.  Balances Act vs PE.
  - finals: reduce on Pool, tiny add/reciprocal on DVE, out-writes on Act
    (Copy scale=drq bias=-drq); last chunk (m=15) runs a minimal all-DVE
    chain reading the scan output directly to shorten the tail.
  - DMA: crowb/x0/wh/cb2/crow2 early on HWDGE, ublk split into 3 pieces,
    xs1 via the gpsimd SWDGE queue in parallel.
"""

import numpy as np

B, F, P, C = 16384, 512, 128, 10
NCORES = 8
BL = B // NCORES          # 2048 rows per core
SEG = P + 1               # 129 columns per class segment
QN = C * SEG              # 1290 scan columns
OUTW = 16 * (C + 1)       # 176 packed output columns

# batch-column slices (start, ncols); chunk m = col/128, 16 chunks total
SLICES = [(0, 128), (128, 128), (256, 256), (512, 512), (1024, 384),
          (1408, 512), (1920, 128)]
ONPE = (0, 2, 4, 6, 8, 10, 12, 14)   # chunks: +1/3 via K=2 crow2 matmul,
                                      # scan reads PSUM (no Act evac)
# per-slice rho-chain mode: "dve" = dent/rec/rho all DVE (pre-stream window)
# "lnexp" = dent on Pool, rec = Exp(-Ln(dent)) on Act, rho mult on Pool
RHO_MODE = ("dve", "dve", "mix", "mix", "mix", "mix", "mix")
DMA_ORDER = ["wh", "xs0", "cb2", "crowb", "crow2p", "ublkAp", "xs1",
             "xs23", "ublkB", "ublkC", "x47", "x811", "x1215"]
FINDVE = (15,)            # chunks with the minimal all-DVE finals tail
BATCH_Q3 = True
LINEARIZE = False
SCAN_BUFS = 3
POOL_MODE = "stack"
SI_BUFS = 4
RHO_BUFS = 5
PIPE_LAG = 3
FIN_REDUCE_POOL = False   # gpsimd tensor_reduce is partition-axis only
WARMUP = 2

_PROG = {}
REPS = 1


def _build_program():
    import concourse.bacc as bacc
    import concourse.bass as bass
    import concourse.tile as tile
    from concourse import bass_isa, mybir

    f32 = mybir.dt.float32
    f16 = mybir.dt.float16
    f32r = mybir.dt.float32r
    Alu = mybir.AluOpType
    Act = mybir.ActivationFunctionType

    nc = bacc.Bacc("TRN2", target_bir_lowering=False, debug=False)

    # x slice-contiguous: 128-col block (slice s, chunk c) at col s*512+c*128
    xq_d = nc.dram_tensor("xq", [128, 8192], f16, kind="ExternalInput").ap()
    wh_d = nc.dram_tensor("whp", [128, 1024], f16, kind="ExternalInput").ap()
    cb2_d = nc.dram_tensor("cb2", [2, BL], f32r, kind="ExternalInput").ap()
    crowb_d = nc.dram_tensor("crowb", [128, 1], f32, kind="ExternalInput").ap()
    ublk_d = nc.dram_tensor("ublk", [P, QN], f16, kind="ExternalInput").ap()
    c2ab_d = nc.dram_tensor("crow2ab", [2, QN], f32r, kind="ExternalInput").ap()
    out_d = nc.dram_tensor("out", [128, OUTW], f32, kind="ExternalOutput").ap()

    with tile.TileContext(nc, linearize=LINEARIZE,
                          pool_alloc_mode=POOL_MODE) as tc:
        for _rep in range(REPS):
            with (
                tc.tile_pool(name="const", bufs=1) as cpool,
                tc.tile_pool(name="xin", bufs=1) as xpool,
                tc.tile_pool(name="mid", bufs=1) as mpool,
                tc.tile_pool(name="scan", bufs=SCAN_BUFS) as spool,
                tc.tile_pool(name="pst", bufs=2, space=bass.MemorySpace.PSUM) as pst,
                tc.tile_pool(name="pq", bufs=2, space=bass.MemorySpace.PSUM) as pq,
            ):
                # ---- input DMAs (HWDGE serializes at 625ns/descriptor; DMA
                #      transfers serialize at ~360B/ns: order = startup path) ----
                xall = xpool.tile([128, 8192], f16, tag="xall")
                wh = cpool.tile([128, 1024], f16, tag="wh")
                cb2 = cpool.tile([2, BL], f32r, tag="cb2")
                crowb = cpool.tile([128, 1], f32, tag="crowb")
                ublk = cpool.tile([P, QN], f16, tag="ublk")
                crow2 = cpool.tile([2, QN], f32r, tag="crow2")

                dmas = {
                    "xs0": lambda: nc.sync.dma_start(xall[:, 0:512], xq_d[:, 0:512]),
                    "xs0p": lambda: nc.gpsimd.dma_start(xall[:, 0:512], xq_d[:, 0:512]),
                    "wh": lambda: nc.sync.dma_start(wh[:], wh_d[:]),
                    "whp": lambda: nc.gpsimd.dma_start(wh[:], wh_d[:]),
                    "cb2": lambda: nc.sync.dma_start(cb2[:], cb2_d[:]),
                    "cb2p": lambda: nc.gpsimd.dma_start(cb2[:], cb2_d[:]),
                    "crowbp": lambda: nc.gpsimd.dma_start(crowb[:], crowb_d[:]),
                    "crow2p": lambda: nc.gpsimd.dma_start(crow2[:], c2ab_d[:]),
                    "crowb": lambda: nc.sync.dma_start(crowb[:], crowb_d[:]),
                    "crow2": lambda: nc.sync.dma_start(crow2[:], c2ab_d[:]),
                    "ublk": lambda: nc.sync.dma_start(ublk[:], ublk_d[:]),
                    "ublkp": lambda: nc.gpsimd.dma_start(ublk[:], ublk_d[:]),
                    "ublkA": lambda: nc.sync.dma_start(ublk[:, 0:512], ublk_d[:, 0:512]),
                    "ublkAp": lambda: nc.gpsimd.dma_start(ublk[:, 0:512], ublk_d[:, 0:512]),
                    "ublkB": lambda: nc.sync.dma_start(ublk[:, 512:1024], ublk_d[:, 512:1024]),
                    "ublkC": lambda: nc.sync.dma_start(ublk[:, 1024:QN], ublk_d[:, 1024:QN]),
                    "xs1": lambda: nc.sync.dma_start(xall[:, 512:1024], xq_d[:, 512:1024]),
                    "xs1p": lambda: nc.gpsimd.dma_start(xall[:, 512:1024], xq_d[:, 512:1024]),
                    "xs23": lambda: nc.sync.dma_start(xall[:, 1024:2048], xq_d[:, 1024:2048]),
                    "x47": lambda: nc.sync.dma_start(xall[:, 2048:4096], xq_d[:, 2048:4096]),
                    "x811": lambda: nc.sync.dma_start(xall[:, 4096:6144], xq_d[:, 4096:6144]),
                    "x1215": lambda: nc.sync.dma_start(xall[:, 6144:8192], xq_d[:, 6144:8192]),
                }
                # ---- device-built constants (memsets precede the Pool
                #      SWDGE issues so nothing queues behind them) ----
                ones2f = cpool.tile([2, 128], f32, tag="ones2")
                nc.gpsimd.memset(ones2f[:], 1.0)
                ones2 = ones2f[:].bitcast(f32r)
                data1 = cpool.tile([128, QN], f32, tag="data1")
                nc.gpsimd.memset(data1[:], 2.0 / 3.0)
                d1v = data1[:].rearrange("p (k s) -> p k s", s=SEG)
                nc.gpsimd.memset(d1v[:, :, 0], 1.0)

                for k in DMA_ORDER:
                    dmas[k]()
                actw = cpool.tile([2, 1], f32, tag="actw")
                nc.scalar.activation(actw[:], ones2f[:, 0:1], Act.Exp)

                nb9 = cpool.tile([128, 1], f32, tag="nb9")
                nc.gpsimd.memset(nb9[:], -(C - 1.0) / C)
                warmb = cpool.tile([2, 512], f32, tag="warmb")
                nc.gpsimd.memset(warmb[:], 1.0)
                warmr = warmb[:].bitcast(f32r)

                zf = mpool.tile([128, 16 * C], f32, tag="zf")
                nsplit = [(0, 512), (512, 512), (1024, QN - 1024)]

                def _xslice(cs, w, c):
                    # x cols for chunk-range [cs, cs+w), weight chunk c
                    s0, n = cs // 128, w // 128
                    v = xall[:].rearrange("p (s c x) -> p s c x", c=4, x=128)
                    return v[:, s0:s0 + n, c, :]

                # ---- finals: y=z-1, dr=1/(sum z - 9), out=z*dr-dr, out[C]=dr
                outq3 = mpool.tile([128, 4 * (C + 1)], f32, tag="outq3")

                def finals(m0, m1, outq):
                    # fully off-DVE: sum(y)-9 via Act accum_out (bias=-0.9
                    # per class column), 1/x via Pool normalize_recip.
                    n = m1 - m0
                    q = m0 // 4
                    drq = mpool.tile([128, n], f32, name=f"dr{m0}",
                                     tag=f"drq{n}", bufs=2)
                    junk = mpool.tile([128, C], f32, name=f"jk{m0}",
                                      tag="junk", bufs=2)
                    junk1 = mpool.tile([128, n], f32, name=f"jk1{m0}",
                                       tag="junk1", bufs=2)
                    for i in range(n):
                        nc.scalar.activation(
                            junk[:], zf[:, C * (m0 + i):C * (m0 + i + 1)],
                            Act.Identity, bias=nb9[:, 0:1],
                            accum_out=drq[:, i:i + 1])
                    for i in range(n):
                        nc.gpsimd.normalize_recip(
                            junk1[:, i:i + 1], drq[:, i:i + 1],
                            drq[:, i:i + 1])
                    ndr = mpool.tile([128, n], f32, name=f"ndr{m0}",
                                     tag=f"ndr{n}", bufs=2)
                    nc.gpsimd.tensor_scalar_mul(ndr[:], drq[:], -1.0)
                    for i in range(n):
                        s = m0 + i - 4 * q
                        nc.scalar.activation(
                            outq[:, (C + 1) * s:(C + 1) * s + C],
                            zf[:, C * (m0 + i):C * (m0 + i + 1)],
                            Act.Identity, scale=drq[:, i:i + 1],
                            bias=ndr[:, i:i + 1])
                    ovv = outq[:].rearrange("p (s k) -> p s k", k=C + 1)
                    nc.gpsimd.tensor_copy(ovv[:, m0 - 4 * q:m1 - 4 * q, C],
                                          drq[:])

                def finals_dve(m, sov, outq, ocol):
                    # minimal-latency all-DVE finals for the tail chunk,
                    # reading the scan output tile directly (no zf staging)
                    yv = sov[:, :, SEG - 1]                   # [128, C] strided
                    szq = mpool.tile([128, 1], f32, name=f"szd{m}", tag="szd")
                    nc.vector.tensor_reduce(szq[:], yv,
                                            axis=mybir.AxisListType.X,
                                            op=Alu.add)
                    nc.vector.tensor_scalar_add(szq[:], szq[:], -(C - 1.0))
                    nc.vector.reciprocal(outq[:, ocol + C:ocol + C + 1], szq[:])
                    dr = outq[:, ocol + C:ocol + C + 1]
                    nc.vector.tensor_scalar(
                        outq[:, ocol:ocol + C], yv,
                        scalar1=dr, scalar2=dr,
                        op0=Alu.mult, op1=Alu.subtract)

                def q_dma(q, outq):
                    nc.sync.dma_start(out_d[:, 44 * q:44 * (q + 1)], outq[:])

                # ---- chunk front: qs matmul -> (+1/3) -> scan ----
                so_map = {}

                def chunk_front(m, rho, joff):
                    on_pe = m in ONPE
                    qs = pq.tile([128, QN], f32, name=f"qs{m}", tag="qs")
                    so = spool.tile([128, QN], f32, name=f"so{m}", tag="so",
                                    bufs=6)
                    so_map[m] = so
                    sh = None
                    if not on_pe:
                        sh = spool.tile([128, QN], f32, name=f"sh{m}",
                                        tag="sh", bufs=3)
                    for (o, n) in nsplit:
                        nc.tensor.matmul(
                            qs[:, o:o + n],
                            rho[:, 128 * joff:128 * (joff + 1)],
                            ublk[:, o:o + n], start=True, stop=not on_pe)
                        if on_pe:
                            nc.tensor.matmul(
                                qs[:, o:o + n], ones2, crow2[:, o:o + n],
                                start=False, stop=True)
                    if on_pe:
                        d0 = qs
                    else:
                        nc.scalar.activation(sh[:], qs[:], Act.Copy,
                                             bias=1.0 / 3.0)
                        d0 = sh
                    nc.vector.tensor_tensor_scan(
                        so[:], d0[:], data1[:], initial=1.0,
                        op0=Alu.mult, op1=Alu.add)

                # ---- chunk post: z extract -> finals/DMA ----
                def fin_chunk(m, yv, outq, s):
                    # per-chunk finals fully off-DVE (yv: [128, C] z values)
                    drq = mpool.tile([128, 1], f32, name=f"dr{m}",
                                     tag="drq1", bufs=4)
                    junk = mpool.tile([128, C], f32, name=f"jk{m}",
                                      tag="junk", bufs=2)
                    junk1 = mpool.tile([128, 1], f32, name=f"jk1{m}",
                                       tag="junk1", bufs=2)
                    ndr = mpool.tile([128, 1], f32, name=f"ndr{m}",
                                     tag="ndr1", bufs=4)
                    nc.scalar.activation(junk[:], yv, Act.Identity,
                                         bias=nb9[:, 0:1],
                                         accum_out=drq[:])
                    nc.gpsimd.normalize_recip(junk1[:], drq[:], drq[:])
                    nc.gpsimd.tensor_scalar_mul(ndr[:], drq[:], -1.0)
                    nc.scalar.activation(
                        outq[:, (C + 1) * s:(C + 1) * s + C], yv,
                        Act.Identity, scale=drq[:, 0:1], bias=ndr[:, 0:1])
                    nc.gpsimd.tensor_copy(
                        outq[:, (C + 1) * s + C:(C + 1) * s + C + 1], drq[:])

                def chunk_post(m):
                    so = so_map.pop(m)
                    sov = so[:].rearrange("p (k s) -> p k s", s=SEG)
                    if m not in FINDVE:
                        nc.gpsimd.tensor_copy(
                            zf[:, C * m:C * (m + 1)], sov[:, :, SEG - 1])
                    # finals: batched per quarter; per-chunk on last quarter
                    if m in (3, 7, 11):
                        q = m // 4
                        oq = mpool.tile([128, 4 * (C + 1)], f32,
                                        name=f"outq{q}", tag="outq", bufs=2)
                        finals(4 * q, 4 * q + 4, oq)
                        q_dma(q, oq)
                    elif m == 14 and BATCH_Q3:
                        finals(12, 15, outq3)
                    elif m == 15:
                        if 15 in FINDVE:
                            finals_dve(15, sov, outq3, 33)
                        else:
                            fin_chunk(15, zf[:, C * 15:C * 16], outq3, 3)
                        nc.sync.dma_start(out_d[:, 132:176], outq3[:])

                # ---- per-slice pipeline (chunk stage lags TWO slices, and
                #      long-latency chain tails (Ln/rec/rho) are emitted
                #      after the older slice's chunk work: engine SEQs issue
                #      in order, so a waiting chain op ahead of a ready evac
                #      or qs matmul would stall the whole stream) ----
                pending = []          # (first_chunk, nchunks, rho_tile)
                flushed = []          # chunk ids with front emitted, post not

                def flush_front():
                    (m0, nch, rr) = pending.pop(0)
                    for j in range(nch):
                        chunk_front(m0 + j, rr, j)
                        flushed.append(m0 + j)

                def flush_posts():
                    while flushed:
                        chunk_post(flushed.pop(0))

                for si_idx, (cs, w) in enumerate(SLICES):
                    mode = RHO_MODE[si_idx]
                    if len(pending) == PIPE_LAG:
                        flush_front()
                    T = pst.tile([128, 512], f32, name=f"T{cs}", tag="T")
                    if si_idx == 0:
                        for _w in range(WARMUP):
                            nc.tensor.matmul(T[:, 0:512], wh[0:2, 0:128],
                                             wh[0:2, 0:512],
                                             start=True, stop=False)
                    for c in range(4):
                        nc.tensor.matmul(T[:, 0:w],
                                         wh[:, 128 * c:128 * (c + 1)],
                                         _xslice(cs, w, c),
                                         start=(c == 0), stop=False)
                    for c in range(4):
                        nc.tensor.matmul(
                            T[:, 0:w],
                            wh[:, 512 + 128 * c:512 + 128 * (c + 1)],
                            _xslice(cs, w, c),
                            start=False, stop=False)
                    nc.tensor.matmul(T[:, 0:w], ones2, cb2[:, cs:cs + w],
                                     start=False, stop=True)

                    si = mpool.tile([128, 512], f32, name=f"si{cs}", tag="si",
                                    bufs=SI_BUFS)
                    nc.scalar.activation(si[:, 0:w], T[:, 0:w], Act.Exp,
                                         bias=crowb[:, 0:1])
                    amax = mpool.tile([128, 512], f32, name=f"am{cs}",
                                      tag="amax", bufs=2)
                    nc.gpsimd.partition_all_reduce(
                        amax[:, 0:w], si[:, 0:w], channels=128,
                        reduce_op=bass_isa.ReduceOp.max)
                    rho = mpool.tile([128, 512], f16, name=f"rho{cs}",
                                     tag="rho", bufs=RHO_BUFS)
                    if mode in ("dve", "mix", "mixp"):
                        dent = mpool.tile([128, 512], f32, name=f"dent{cs}",
                                          tag="dentv", bufs=2)
                        if mode == "mixp":
                            d0p = mpool.tile([128, 512], f32, name=f"d0{cs}",
                                             tag="d0p", bufs=2)
                            nc.gpsimd.tensor_sub(d0p[:, 0:w], amax[:, 0:w],
                                                 si[:, 0:w])
                            nc.gpsimd.tensor_scalar_add(dent[:, 0:w],
                                                        d0p[:, 0:w], 1e-4)
                        else:
                            nc.vector.scalar_tensor_tensor(
                                dent[:, 0:w], amax[:, 0:w], 1e-4, si[:, 0:w],
                                op0=Alu.add, op1=Alu.subtract)
                        rec = mpool.tile([128, 512], f32, name=f"rec{cs}",
                                         tag="rec", bufs=2)
                        nc.vector.reciprocal_approx_fast(rec[:, 0:w],
                                                         dent[:, 0:w])
                        if mode == "dve":
                            nc.vector.tensor_mul(rho[:, 0:w], si[:, 0:w],
                                                 rec[:, 0:w])
                        else:
                            nc.gpsimd.tensor_mul(rho[:, 0:w], si[:, 0:w],
                                                 rec[:, 0:w])
                    else:
                        d0p = mpool.tile([128, 512], f32, name=f"d0{cs}",
                                         tag="d0p", bufs=2)
                        nc.gpsimd.tensor_sub(d0p[:, 0:w], amax[:, 0:w],
                                             si[:, 0:w])
                        dent = mpool.tile([128, 512], f32, name=f"dent{cs}",
                                          tag="dentp", bufs=2)
                        nc.gpsimd.tensor_scalar_add(dent[:, 0:w], d0p[:, 0:w],
                                                    1e-4)
                        ldt = mpool.tile([128, 512], f32, name=f"ld{cs}",
                                         tag="ldt", bufs=2)
                        nc.scalar.activation(ldt[:, 0:w], dent[:, 0:w], Act.Ln)
                        rec = mpool.tile([128, 512], f32, name=f"rec{cs}",
                                         tag="rec", bufs=2)
                        nc.scalar.activation(rec[:, 0:w], ldt[:, 0:w], Act.Exp,
                                             scale=-1.0)
                        nc.gpsimd.tensor_mul(rho[:, 0:w], si[:, 0:w],
                                             rec[:, 0:w])
                    flush_posts()
                    pending.append((cs // 128, w // 128, rho))
                while pending:
                    flush_front()
                    flush_posts()

    nc.compile()
    # The act-table insertion pass picks tables greedily per function (Exp ->
    # exp_and_others id 0, Ln -> natural_log id 5) and thrashes 1283ns loads
    # between them.  Table 6 (natural_log_exp_and_others) contains Exp, Ln,
    # Copy and Identity together: re-point every load at it and drop the
    # now-redundant swaps (only ones carrying no semaphore info).
    for blk in nc.main_func.blocks:
        keep = []
        loaded = False
        for inst in blk.instructions:
            if isinstance(inst, mybir.InstLoadActFuncSet) \
                    and inst.act_func_set_id in (0, 5, 6):
                si = getattr(inst, "sync_info", None)
                empty = si is None or (not si.on_wait and not si.on_update)
                if loaded and empty:
                    continue
                inst.act_func_set_id = 6
                loaded = True
            keep.append(inst)
        blk.instructions[:] = keep
    return nc


def _f32r_round(v):
    # float32r = RNE to 11 explicit mantissa bits (HW-verified).
    u = np.asarray(v, np.float32).view(np.uint32).astype(np.uint64)
    drop = 12
    half = np.uint64(1 << (drop - 1))
    odd = (u >> np.uint64(drop)) & np.uint64(1)
    u2 = (u + half - np.uint64(1) + odd) & np.uint64(~((1 << drop) - 1) & 0xFFFFFFFF)
    return u2.astype(np.uint32).view(np.float32)


def _host_prep(x, w, eta, xi, beta):
    """Host-side: shard/layout x, build tiny replicated param matrices."""
    x = np.asarray(x, np.float32)
    w = np.asarray(w, np.float32)
    eta = np.asarray(eta, np.float32).reshape(-1)
    xi = np.asarray(xi, np.float32).reshape(-1)
    beta = np.asarray(beta, np.float32)

    gamma = (eta.astype(np.float64)) ** 2                # [P]
    if np.ptp(gamma) != 0.0:
        raise NotImplementedError(
            "kernel assumes per-prototype-constant gamma (eta); the shipped "
            "problem uses eta = full(0.1)")
    g0 = float(gamma[0])
    alpha = 1.0 / (1.0 + np.exp(-xi.astype(np.float64)))
    wsq = (w.astype(np.float64) ** 2).sum(-1)            # [P]

    wt2g = 2.0 * gamma[None, :] * w.T.astype(np.float64)   # [F,P] f64
    whb = wt2g.astype(np.float16)                        # [F, P] fp16 hi
    wrb = (wt2g - whb.astype(np.float64)).astype(np.float16)  # fp16 residual
    whp = np.zeros((128, 1024), np.float16)
    for c in range(4):
        whp[:, 128 * c:128 * (c + 1)] = whb[128 * c:128 * (c + 1), :]
        whp[:, 512 + 128 * c:512 + 128 * (c + 1)] = wrb[128 * c:128 * (c + 1), :]

    crow_bias = (np.log(alpha) - gamma * wsq).astype(np.float32)[:, None]  # [P,1]

    b2 = beta.astype(np.float64) ** 2
    u = b2 / b2.sum(0, keepdims=True)                    # [C,P]
    uh = u / 3.0
    third_a = float(_f32r_round(np.float32(1.0 / 3.0)))
    third_b = np.float32(1.0 / 3.0 - third_a)
    ublk = np.zeros((P, QN), np.float16)
    crow2ab = np.zeros((2, QN), np.float32)
    for k in range(C):
        base = k * SEG
        crow2ab[0, base + 1:base + SEG] = third_a
        crow2ab[1, base + 1:base + SEG] = third_b
        for t in range(P):
            v = uh[k, t] * (3.0 if t == 0 else 1.0)
            ublk[t, base + 1 + t] = np.float16(v)

    shards = x.reshape(NCORES, BL, F)
    in_maps = []
    for i in range(NCORES):
        xs = shards[i]                                    # [BL, F] f32
        xt = np.ascontiguousarray(xs.T)                   # [F, BL]
        xh = xt.astype(np.float16)
        # slice-contiguous: block (slice s of 16, chunk c) at col s*512+c*128
        xqp = np.zeros((128, 8192), np.float16)
        for s in range(16):
            for c in range(4):
                xqp[:, 512 * s + 128 * c:512 * s + 128 * (c + 1)] = \
                    xh[128 * c:128 * (c + 1), 128 * s:128 * (s + 1)]
        # -g|x|^2 per batch row, f64 -> f32r a + f32r b residual rows
        cb = -(g0 * (xs.astype(np.float64) ** 2).sum(-1))          # [BL]
        cba = _f32r_round(cb.astype(np.float32))
        cbb = _f32r_round((cb - cba.astype(np.float64)).astype(np.float32))
        cb2 = np.stack([cba, cbb], axis=0)                         # [2, BL]
        in_maps.append({
            "xq": xqp, "whp": whp, "cb2": cb2, "crowb": crow_bias,
            "ublk": ublk, "crow2ab": crow2ab,
        })
    return in_maps


def _run(in_maps, trace=False):
    from concourse.bass_utils import run_bass_kernel_spmd

    if "nc" not in _PROG:
        _PROG["nc"] = _build_program()
    nc = _PROG["nc"]
    res = run_bass_kernel_spmd(
        nc, in_maps, core_ids=list(range(NCORES)), trace=trace)
    outs = []
    for i in range(NCORES):
        o = np.asarray(res.results[i]["out"])          # [128, 176]
        outs.append(o.reshape(128, 16, C + 1).transpose(1, 0, 2).reshape(BL, C + 1))
    full = np.concatenate(outs, axis=0).astype(np.float32)
    return full, res


def kernel(x, w, eta, xi, beta):
    in_maps = _host_prep(x, w, eta, xi, beta)
    full, _ = _run(in_maps, trace=False)
    return full
